# revision 8
# baseline (speedup 1.0000x reference)
"""Trainium2 Bass kernel for per-head bilinear graph attention.

Reference computation (B=4, N=2048, IN=256, H=8, ATN=32):
    xt     = einsum('bni,hio->bhno', x, W) + b          # [B,H,N,32]
    xC     = einsum('bhno,hpo->bhnp', xt, C)            # [B,H,N,32]
    scores = einsum('bhnp,bhmp->bhnm', xC, xt)          # [B,H,N,N]
    alpha  = tanh(scores * adj[:,None])                 # [B,H,N,N]
    heads  = einsum('bhnm,bhmo->bhno', alpha, xt)       # [B,H,N,32]
    out    = concat heads on feature dim                # [B,N,256]

Sharding: 8 cores = 4 batches x 2 head-groups (4 heads each). Fully
data-parallel, no collectives. Each core computes out[b, :, hg*128:(hg+1)*128]
transposed ([128, 2048]); the host transposes back and concatenates.

Device-side layout is fully transposed ("T" = [feature/m, n]):
    xtT  [128(4h x 32o), 2048n]   stacked per-head xt^T (bias included)
    xCT  [128(4h x 32p), 2048n]   stacked per-head xC^T
    sT   [128m, n]     = scores[n, m]   (psum, per m-chunk per head)
    z    = sT * adjT   (adjT host-pretransposed so it is [m, n])
    alphaT = tanh(z)
    outT [128(4h x 32o), 2048n] accumulated in psum over 16 m-chunks

Engine budget per core (predicted): DVE mult ~153us (bottleneck), ACT
tanh ~125us, PE ~45us, DMA ~31us.
"""

import sys
import types

import numpy as np


def _ensure_axon_ntff_hook():
    """Provide antenv.axon_hooks if the image lacks it, so
    run_bass_kernel_spmd(trace=True) can capture NTFF profiles instead of
    crashing on the import. No-op when the real module exists."""
    try:
        import antenv.axon_hooks  # noqa: F401

        return
    except ImportError:
        pass
    mod = types.ModuleType("antenv.axon_hooks")
    _state = {"hook": None}
    mod.set_axon_ntff_profile_hook = lambda h: _state.__setitem__("hook", h)
    mod.get_axon_ntff_profile_hook = lambda: _state["hook"]
    sys.modules["antenv.axon_hooks"] = mod
    try:
        import antenv

        antenv.axon_hooks = mod
    except ImportError:
        pass
    try:
        from trn_agent_boot.trn_boot import _ntff_profile_via_ctypes

        mod.set_axon_ntff_profile_hook(
            _ntff_profile_via_ctypes("/opt/axon/libaxon_pjrt.so")
        )
    except Exception:
        pass


_ensure_axon_ntff_hook()

from concourse import bacc, mybir, tile
import concourse.bass as bass
from concourse.bass_utils import run_bass_kernel_spmd
from concourse.masks import make_identity

F32 = mybir.dt.float32
AF = mybir.ActivationFunctionType
ALU = mybir.AluOpType

P = 128
B, N, IN_DIM, H, ATN = 4, 2048, 256, 8, 32
NH = 4                # heads per core
NCORES = 8
MC = N // P           # 16 m-chunks
IC = IN_DIM // P      # 2 contraction chunks for the input projection

_CACHE = {}


def build_graph():
    nc = bacc.Bacc("TRN2", target_bir_lowering=False, debug=False)

    xT_d = nc.dram_tensor("xT", [IN_DIM, N], F32, kind="ExternalInput")
    adjT_d = nc.dram_tensor("adjT", [N, N], F32, kind="ExternalInput")
    W_d = nc.dram_tensor("Wt", [P, IC, NH, ATN], F32, kind="ExternalInput")
    CT_d = nc.dram_tensor("CTt", [P, ATN], F32, kind="ExternalInput")
    b_d = nc.dram_tensor("bias", [P, 1], F32, kind="ExternalInput")
    out_d = nc.dram_tensor("out", [P, N], F32, kind="ExternalOutput")

    with tile.TileContext(nc) as tc:
        with (
            tc.tile_pool(name="const", bufs=1) as cp,
            tc.tile_pool(name="adj", bufs=3) as adjp,
            tc.tile_pool(name="z", bufs=2) as zp,
            tc.tile_pool(name="alpha", bufs=1) as alp,
            tc.tile_pool(name="ps_o", bufs=1, space="PSUM") as ps_o,
            tc.tile_pool(name="ps_s", bufs=2, space="PSUM") as ps_s,
        ):
            ident = cp.tile([P, P], F32)
            make_identity(nc, ident[:])

            xT_sb = cp.tile([P, IC, N], F32)
            nc.sync.dma_start(xT_sb[:], xT_d[:].rearrange("(c p) n -> p c n", p=P))
            W_sb = cp.tile([P, IC, NH, ATN], F32)
            nc.sync.dma_start(W_sb[:], W_d[:])
            CT_sb = cp.tile([P, ATN], F32)
            nc.sync.dma_start(CT_sb[:], CT_d[:])
            b_sb = cp.tile([P, 1], F32)
            nc.sync.dma_start(b_sb[:], b_d[:])

            xtT = cp.tile([P, N], F32)
            xCT = cp.tile([P, N], F32)
            xt4 = cp.tile([P, MC, P], F32)
            out_sb = cp.tile([P, N], F32)
            zrow = cp.tile([1, 512], F32)
            nc.vector.memset(zrow[:], 0.0)

            # --- prologue (reuses the ps_s slots) ---
            # xtT[32h+o, n] = sum_i W[h,i,o] x[n,i] + b[h,o]
            # col-tiled: 4 heads stacked on output partitions.
            for nq in range(N // 512):
                pt = ps_s.tile([P, 1024], F32, tag="s")
                # head-outer, chunk-inner: each head's accumulation chain
                # completes before the next head's start=True (safe even if
                # the HW first-matmul clear is bank-wide across partitions).
                for h in range(NH):
                    for c in range(IC):
                        nc.tensor.matmul(
                            pt[bass.ts(h, ATN), :512],
                            W_sb[:, c, h, :],
                            xT_sb[:, c, bass.ts(nq, 512)],
                            start=(c == 0),
                            stop=(c == IC - 1),
                            tile_position=(0, h * ATN),
                            skip_group_check=True,
                        )
                nc.scalar.activation(
                    xtT[:, bass.ts(nq, 512)], pt[:, :512], AF.Identity, bias=b_sb[:]
                )

            # xCT[32h+p, n] = sum_o C[h,p,o] xt[n,o]; diagonal 32x32 tiles.
            for nq in range(N // 512):
                pt = ps_s.tile([P, 1024], F32, tag="s")
                for h in range(NH):
                    nc.tensor.matmul(
                        pt[bass.ts(h, ATN), :512],
                        CT_sb[bass.ts(h, ATN), :],
                        xtT[bass.ts(h, ATN), bass.ts(nq, 512)],
                        start=True,
                        stop=True,
                        tile_position=(h * ATN, h * ATN),
                        skip_group_check=True,
                    )
                nc.scalar.copy(xCT[:, bass.ts(nq, 512)], pt[:, :512])

            # xt4[m_local, mc, f] = xt[mc*128+m_local, f]  (PE transpose)
            for mc in range(MC):
                pt = ps_s.tile([P, 1024], F32, tag="s")
                nc.tensor.transpose(pt[:, :P], xtT[:, bass.ts(mc, P)], ident[:])
                nc.scalar.copy(xt4[:, mc, :], pt[:, :P])

            po = ps_o.tile([P, N], F32)
            # Seed each output-accumulator bank with an explicit zeroing
            # matmul (K=1, zero weights) so every real outT matmul can use
            # start=False: correct regardless of whether the HW first-matmul
            # has_written clear is per-partition-slice or bank-wide.
            for q in range(N // 512):
                nc.tensor.matmul(
                    po[:, bass.ts(q, 512)],
                    zrow[:, :P],
                    zrow[:, :512],
                    start=True,
                    stop=False,
                    skip_group_check=True,
                )

            # --- main loop over m-chunks ---
            for mc in range(MC):
                adjt = adjp.tile([P, N], F32, tag="adj")
                nc.sync.dma_start(adjt[:], adjT_d[bass.ts(mc, P), :])
                zb = zp.tile([P, NH, N], F32, tag="z")
                for h in range(NH):
                    for nh in range(N // 1024):
                        sp = ps_s.tile([P, 1024], F32, tag="s")
                        for q in range(2):
                            nc.tensor.matmul(
                                sp[:, bass.ts(q, 512)],
                                xtT[bass.ts(h, ATN), bass.ts(mc, P)],
                                xCT[bass.ts(h, ATN), bass.ds(nh * 1024 + q * 512, 512)],
                                start=True,
                                stop=True,
                                tile_position=(h * ATN, 0),
                            )
                        nc.vector.tensor_tensor(
                            zb[:, h, bass.ds(nh * 1024, 1024)],
                            sp[:],
                            adjt[:, bass.ds(nh * 1024, 1024)],
                            ALU.mult,
                        )
                alpha = alp.tile([P, NH, N], F32, tag="alpha")
                for hp in range(2):
                    nc.scalar.activation(
                        alpha[:, bass.ds(hp * 2, 2), :],
                        zb[:, bass.ds(hp * 2, 2), :],
                        AF.Tanh,
                    )
                for q in range(N // 512):
                    for h in range(NH):
                        nc.tensor.matmul(
                            po[bass.ts(h, ATN), bass.ts(q, 512)],
                            xt4[:, mc, bass.ts(h, ATN)],
                            alpha[:, h, bass.ts(q, 512)],
                            start=False,
                            stop=(mc == MC - 1 and h == NH - 1),
                            tile_position=(0, h * ATN),
                            skip_group_check=True,
                        )

            nc.scalar.copy(out_sb[:], po[:])
            nc.sync.dma_start(out_d[:], out_sb[:])

    nc.compile()
    return nc


def _get_graph():
    if "nc" not in _CACHE:
        _CACHE["nc"] = build_graph()
    return _CACHE["nc"]


def make_in_maps(x, adj, W, b, C):
    in_maps = []
    for core in range(NCORES):
        bb = core // 2
        hg = core % 2
        hs = slice(hg * NH, (hg + 1) * NH)
        in_maps.append(
            {
                "xT": np.ascontiguousarray(x[bb].T),
                "adjT": np.ascontiguousarray(adj[bb].T),
                "Wt": np.ascontiguousarray(
                    W[hs].reshape(NH, IC, P, ATN).transpose(2, 1, 0, 3)
                ),
                "CTt": np.ascontiguousarray(
                    C[hs].transpose(0, 2, 1).reshape(NH * ATN, ATN)
                ),
                "bias": np.ascontiguousarray(b[hs].reshape(P, 1)),
            }
        )
    return in_maps


LAST_RESULT = None


def kernel(x, adj, W, b, C):
    global LAST_RESULT
    x = np.asarray(x, dtype=np.float32)
    adj = np.asarray(adj, dtype=np.float32)
    W = np.asarray(W, dtype=np.float32)
    b = np.asarray(b, dtype=np.float32)
    C = np.asarray(C, dtype=np.float32)

    nc = _get_graph()
    in_maps = make_in_maps(x, adj, W, b, C)
    res = run_bass_kernel_spmd(nc, in_maps, core_ids=list(range(NCORES)))
    LAST_RESULT = res

    out = np.empty((B, N, H * ATN), dtype=np.float32)
    for core in range(NCORES):
        bb = core // 2
        hg = core % 2
        out[bb, :, hg * P : (hg + 1) * P] = res.results[core]["out"].T
    return out


# revision 10
# speedup vs baseline: 2.2339x; 2.2339x over previous
"""Trainium2 Bass kernel for per-head bilinear graph attention.

Reference computation (B=4, N=2048, IN=256, H=8, ATN=32):
    xt     = einsum('bni,hio->bhno', x, W) + b          # [B,H,N,32]
    xC     = einsum('bhno,hpo->bhnp', xt, C)            # [B,H,N,32]
    scores = einsum('bhnp,bhmp->bhnm', xC, xt)          # [B,H,N,N]
    alpha  = tanh(scores * adj[:,None])                 # [B,H,N,N]
    heads  = einsum('bhnm,bhmo->bhno', alpha, xt)       # [B,H,N,32]
    out    = concat heads on feature dim                # [B,N,256]

Sharding: 8 cores = 4 batches x 2 head-groups (4 heads each). Fully
data-parallel, no collectives. Each core computes out[b, :, hg*128:(hg+1)*128]
transposed ([128, 2048]); the host transposes back and concatenates.

Device-side layout is fully transposed ("T" = [feature/m, n]):
    xtT  [128(4h x 32o), 2048n]   stacked per-head xt^T (bias included)
    xCT  [128(4h x 32p), 2048n]   stacked per-head xC^T
    sT   [128m, n]     = scores[n, m]   (psum, per m-chunk per head)
    z    = sT * adjT   (adjT host-pretransposed so it is [m, n])
    alphaT = tanh(z)
    outT [128(4h x 32o), 2048n] accumulated in psum over 16 m-chunks

Engine budget per core (predicted): DVE mult ~153us (bottleneck), ACT
tanh ~125us, PE ~45us, DMA ~31us.
"""

import sys
import types

import numpy as np
import ml_dtypes

BF16_NP = ml_dtypes.bfloat16


def _ensure_axon_ntff_hook():
    """Provide antenv.axon_hooks if the image lacks it, so
    run_bass_kernel_spmd(trace=True) can capture NTFF profiles instead of
    crashing on the import. No-op when the real module exists."""
    try:
        import antenv.axon_hooks  # noqa: F401

        return
    except ImportError:
        pass
    mod = types.ModuleType("antenv.axon_hooks")
    _state = {"hook": None}
    mod.set_axon_ntff_profile_hook = lambda h: _state.__setitem__("hook", h)
    mod.get_axon_ntff_profile_hook = lambda: _state["hook"]
    sys.modules["antenv.axon_hooks"] = mod
    try:
        import antenv

        antenv.axon_hooks = mod
    except ImportError:
        pass
    try:
        from trn_agent_boot.trn_boot import _ntff_profile_via_ctypes

        mod.set_axon_ntff_profile_hook(
            _ntff_profile_via_ctypes("/opt/axon/libaxon_pjrt.so")
        )
    except Exception:
        pass


_ensure_axon_ntff_hook()

from concourse import bacc, mybir, tile
import concourse.bass as bass
from concourse.bass_utils import run_bass_kernel_spmd
from concourse.masks import make_identity

F32 = mybir.dt.float32
BF16 = mybir.dt.bfloat16
AF = mybir.ActivationFunctionType
ALU = mybir.AluOpType

P = 128
B, N, IN_DIM, H, ATN = 4, 2048, 256, 8, 32
NH = 4                # heads per core
NCORES = 8
MC = N // P           # 16 m-chunks
IC = IN_DIM // P      # 2 contraction chunks for the input projection

_CACHE = {}


def build_graph():
    nc = bacc.Bacc("TRN2", target_bir_lowering=False, debug=False)

    xT_d = nc.dram_tensor("xT", [IN_DIM, N], BF16, kind="ExternalInput")
    adjT_d = nc.dram_tensor("adjT", [N, N], F32, kind="ExternalInput")
    W_d = nc.dram_tensor("Wt", [P, IC, NH, ATN], BF16, kind="ExternalInput")
    CT_d = nc.dram_tensor("CTt", [P, ATN], BF16, kind="ExternalInput")
    b_d = nc.dram_tensor("bias", [P, 1], F32, kind="ExternalInput")
    out_d = nc.dram_tensor("out", [P, N], F32, kind="ExternalOutput")

    with tile.TileContext(nc) as tc:
        with (
            tc.tile_pool(name="const", bufs=1) as cp,
            tc.tile_pool(name="adj", bufs=3) as adjp,
            tc.tile_pool(name="z", bufs=2) as zp,
            tc.tile_pool(name="alpha", bufs=1) as alp,
            tc.tile_pool(name="ps_o", bufs=1, space="PSUM") as ps_o,
            tc.tile_pool(name="ps_s", bufs=2, space="PSUM") as ps_s,
        ):
            ident = cp.tile([P, P], BF16)
            make_identity(nc, ident[:])

            xT_sb = cp.tile([P, IC, N], BF16)
            nc.sync.dma_start(xT_sb[:], xT_d[:].rearrange("(c p) n -> p c n", p=P))
            W_sb = cp.tile([P, IC, NH, ATN], BF16)
            nc.sync.dma_start(W_sb[:], W_d[:])
            CT_sb = cp.tile([P, ATN], BF16)
            nc.sync.dma_start(CT_sb[:], CT_d[:])
            b_sb = cp.tile([P, 1], F32)
            nc.sync.dma_start(b_sb[:], b_d[:])

            xtT = cp.tile([P, N], BF16)
            xCT = cp.tile([P, N], BF16)
            xt4 = cp.tile([P, MC, P], BF16)
            out_sb = cp.tile([P, N], F32)
            zrow = cp.tile([1, 512], BF16)
            nc.vector.memset(zrow[:], 0.0)

            # --- prologue (reuses the ps_s slots) ---
            # xtT[32h+o, n] = sum_i W[h,i,o] x[n,i] + b[h,o]
            # col-tiled: 4 heads stacked on output partitions.
            for nq in range(N // 512):
                pt = ps_s.tile([P, 1024], F32, tag="s")
                # head-outer, chunk-inner: each head's accumulation chain
                # completes before the next head's start=True (safe even if
                # the HW first-matmul clear is bank-wide across partitions).
                for h in range(NH):
                    for c in range(IC):
                        nc.tensor.matmul(
                            pt[bass.ts(h, ATN), :512],
                            W_sb[:, c, h, :],
                            xT_sb[:, c, bass.ts(nq, 512)],
                            start=(c == 0),
                            stop=(c == IC - 1),
                            tile_position=(0, h * ATN),
                            skip_group_check=True,
                        )
                nc.scalar.activation(
                    xtT[:, bass.ts(nq, 512)], pt[:, :512], AF.Identity, bias=b_sb[:]
                )

            # xCT[32h+p, n] = sum_o C[h,p,o] xt[n,o]; diagonal 32x32 tiles.
            for nq in range(N // 512):
                pt = ps_s.tile([P, 1024], F32, tag="s")
                for h in range(NH):
                    nc.tensor.matmul(
                        pt[bass.ts(h, ATN), :512],
                        CT_sb[bass.ts(h, ATN), :],
                        xtT[bass.ts(h, ATN), bass.ts(nq, 512)],
                        start=True,
                        stop=True,
                        tile_position=(h * ATN, h * ATN),
                        skip_group_check=True,
                    )
                nc.scalar.copy(xCT[:, bass.ts(nq, 512)], pt[:, :512])

            # xt4[m_local, mc, f] = xt[mc*128+m_local, f]  (PE transpose)
            for mc in range(MC):
                pt = ps_s.tile([P, 1024], BF16, tag="s")
                nc.tensor.transpose(pt[:, :P], xtT[:, bass.ts(mc, P)], ident[:])
                nc.scalar.copy(xt4[:, mc, :], pt[:, :P])

            po = ps_o.tile([P, N], F32)
            # Seed each output-accumulator bank with an explicit zeroing
            # matmul (K=1, zero weights) so every real outT matmul can use
            # start=False: correct regardless of whether the HW first-matmul
            # has_written clear is per-partition-slice or bank-wide.
            for q in range(N // 512):
                nc.tensor.matmul(
                    po[:, bass.ts(q, 512)],
                    zrow[:, :P],
                    zrow[:, :512],
                    start=True,
                    stop=False,
                    skip_group_check=True,
                )

            # --- main loop over m-chunks ---
            for mc in range(MC):
                adjt = adjp.tile([P, N], F32, tag="adj")
                nc.sync.dma_start(adjt[:], adjT_d[bass.ts(mc, P), :])
                zb = zp.tile([P, NH, N], F32, tag="z")
                for h in range(NH):
                    for nh in range(N // 1024):
                        sp = ps_s.tile([P, 1024], F32, tag="s")
                        for q in range(2):
                            nc.tensor.matmul(
                                sp[:, bass.ts(q, 512)],
                                xtT[bass.ts(h, ATN), bass.ts(mc, P)],
                                xCT[bass.ts(h, ATN), bass.ds(nh * 1024 + q * 512, 512)],
                                start=True,
                                stop=True,
                                tile_position=(h * ATN, 0),
                            )
                        nc.vector.tensor_tensor(
                            zb[:, h, bass.ds(nh * 1024, 1024)],
                            sp[:],
                            adjt[:, bass.ds(nh * 1024, 1024)],
                            ALU.mult,
                        )
                alpha = alp.tile([P, NH, N], BF16, tag="alpha")
                for hp in range(2):
                    nc.scalar.activation(
                        alpha[:, bass.ds(hp * 2, 2), :],
                        zb[:, bass.ds(hp * 2, 2), :],
                        AF.Tanh,
                    )
                for q in range(N // 512):
                    for h in range(NH):
                        nc.tensor.matmul(
                            po[bass.ts(h, ATN), bass.ts(q, 512)],
                            xt4[:, mc, bass.ts(h, ATN)],
                            alpha[:, h, bass.ts(q, 512)],
                            start=False,
                            stop=(mc == MC - 1 and h == NH - 1),
                            tile_position=(0, h * ATN),
                            skip_group_check=True,
                        )

            nc.scalar.copy(out_sb[:], po[:])
            nc.sync.dma_start(out_d[:], out_sb[:])

    nc.compile()
    return nc


def _get_graph():
    if "nc" not in _CACHE:
        _CACHE["nc"] = build_graph()
    return _CACHE["nc"]


def make_in_maps(x, adj, W, b, C):
    in_maps = []
    for core in range(NCORES):
        bb = core // 2
        hg = core % 2
        hs = slice(hg * NH, (hg + 1) * NH)
        in_maps.append(
            {
                "xT": np.ascontiguousarray(x[bb].T).astype(BF16_NP),
                "adjT": np.ascontiguousarray(adj[bb].T),
                "Wt": np.ascontiguousarray(
                    W[hs].reshape(NH, IC, P, ATN).transpose(2, 1, 0, 3)
                ).astype(BF16_NP),
                "CTt": np.ascontiguousarray(
                    C[hs].transpose(0, 2, 1).reshape(NH * ATN, ATN)
                ).astype(BF16_NP),
                "bias": np.ascontiguousarray(b[hs].reshape(P, 1)),
            }
        )
    return in_maps


LAST_RESULT = None


def kernel(x, adj, W, b, C):
    global LAST_RESULT
    x = np.asarray(x, dtype=np.float32)
    adj = np.asarray(adj, dtype=np.float32)
    W = np.asarray(W, dtype=np.float32)
    b = np.asarray(b, dtype=np.float32)
    C = np.asarray(C, dtype=np.float32)

    nc = _get_graph()
    in_maps = make_in_maps(x, adj, W, b, C)
    res = run_bass_kernel_spmd(nc, in_maps, core_ids=list(range(NCORES)))
    LAST_RESULT = res

    out = np.empty((B, N, H * ATN), dtype=np.float32)
    for core in range(NCORES):
        bb = core // 2
        hg = core % 2
        out[bb, :, hg * P : (hg + 1) * P] = res.results[core]["out"].T
    return out


# revision 11
# speedup vs baseline: 2.7231x; 1.2190x over previous
"""Trainium2 Bass kernel for per-head bilinear graph attention.

Reference computation (B=4, N=2048, IN=256, H=8, ATN=32):
    xt     = einsum('bni,hio->bhno', x, W) + b          # [B,H,N,32]
    xC     = einsum('bhno,hpo->bhnp', xt, C)            # [B,H,N,32]
    scores = einsum('bhnp,bhmp->bhnm', xC, xt)          # [B,H,N,N]
    alpha  = tanh(scores * adj[:,None])                 # [B,H,N,N]
    heads  = einsum('bhnm,bhmo->bhno', alpha, xt)       # [B,H,N,32]
    out    = concat heads on feature dim                # [B,N,256]

Sharding: 8 cores = 4 batches x 2 head-groups (4 heads each). Fully
data-parallel, no collectives. Each core computes out[b, :, hg*128:(hg+1)*128]
transposed ([128, 2048]); the host transposes back and concatenates.

Device-side layout is fully transposed ("T" = [feature/m, n]):
    xtT  [128(4h x 32o), 2048n]   stacked per-head xt^T (bias included)
    xCT  [128(4h x 32p), 2048n]   stacked per-head xC^T
    sT   [128m, n]     = scores[n, m]   (psum, per m-chunk per head)
    z    = sT * adjT   (adjT host-pretransposed so it is [m, n])
    alphaT = tanh(z)
    outT [128(4h x 32o), 2048n] accumulated in psum over 16 m-chunks

Engine budget per core (predicted): DVE mult ~153us (bottleneck), ACT
tanh ~125us, PE ~45us, DMA ~31us.
"""

import sys
import types

import numpy as np
import ml_dtypes

BF16_NP = ml_dtypes.bfloat16


def _ensure_axon_ntff_hook():
    """Provide antenv.axon_hooks if the image lacks it, so
    run_bass_kernel_spmd(trace=True) can capture NTFF profiles instead of
    crashing on the import. No-op when the real module exists."""
    try:
        import antenv.axon_hooks  # noqa: F401

        return
    except ImportError:
        pass
    mod = types.ModuleType("antenv.axon_hooks")
    _state = {"hook": None}
    mod.set_axon_ntff_profile_hook = lambda h: _state.__setitem__("hook", h)
    mod.get_axon_ntff_profile_hook = lambda: _state["hook"]
    sys.modules["antenv.axon_hooks"] = mod
    try:
        import antenv

        antenv.axon_hooks = mod
    except ImportError:
        pass
    try:
        from trn_agent_boot.trn_boot import _ntff_profile_via_ctypes

        mod.set_axon_ntff_profile_hook(
            _ntff_profile_via_ctypes("/opt/axon/libaxon_pjrt.so")
        )
    except Exception:
        pass


_ensure_axon_ntff_hook()

from concourse import bacc, mybir, tile
import concourse.bass as bass
from concourse.bass_utils import run_bass_kernel_spmd
from concourse.masks import make_identity

F32 = mybir.dt.float32
BF16 = mybir.dt.bfloat16
AF = mybir.ActivationFunctionType
ALU = mybir.AluOpType

P = 128
B, N, IN_DIM, H, ATN = 4, 2048, 256, 8, 32
NH = 4                # heads per core
NCORES = 8
MC = N // P           # 16 m-chunks
IC = IN_DIM // P      # 2 contraction chunks for the input projection

_CACHE = {}


def build_graph():
    nc = bacc.Bacc("TRN2", target_bir_lowering=False, debug=False)

    xT_d = nc.dram_tensor("xT", [IN_DIM, N], BF16, kind="ExternalInput")
    adjT_d = nc.dram_tensor("adjT", [N, N], F32, kind="ExternalInput")
    W_d = nc.dram_tensor("Wt", [P, IC, NH, ATN], BF16, kind="ExternalInput")
    CT_d = nc.dram_tensor("CTt", [P, ATN], BF16, kind="ExternalInput")
    b_d = nc.dram_tensor("bias", [P, 1], F32, kind="ExternalInput")
    out_d = nc.dram_tensor("out", [P, N], F32, kind="ExternalOutput")

    with tile.TileContext(nc) as tc:
        with (
            tc.tile_pool(name="const", bufs=1) as cp,
            tc.tile_pool(name="adj", bufs=4) as adjp,
            tc.tile_pool(name="z", bufs=3) as zp,
            tc.tile_pool(name="alpha", bufs=2) as alp,
            tc.tile_pool(name="ps_o", bufs=1, space="PSUM") as ps_o,
            tc.tile_pool(name="ps_s", bufs=3, space="PSUM") as ps_s,
        ):
            ident = cp.tile([P, P], BF16)
            make_identity(nc, ident[:])

            xT_sb = cp.tile([P, IC, N], BF16)
            nc.sync.dma_start(xT_sb[:], xT_d[:].rearrange("(c p) n -> p c n", p=P))
            W_sb = cp.tile([P, IC, NH, ATN], BF16)
            nc.sync.dma_start(W_sb[:], W_d[:])
            CT_sb = cp.tile([P, ATN], BF16)
            nc.sync.dma_start(CT_sb[:], CT_d[:])
            b_sb = cp.tile([P, 1], F32)
            nc.sync.dma_start(b_sb[:], b_d[:])

            xtT = cp.tile([P, N], BF16)
            xCT = cp.tile([P, N], BF16)
            xt4 = cp.tile([P, MC, P], BF16)
            out_sb = cp.tile([P, N], F32)
            zrow = cp.tile([1, 512], BF16)
            nc.vector.memset(zrow[:], 0.0)

            # --- prologue (reuses the ps_s slots) ---
            # xtT[32h+o, n] = sum_i W[h,i,o] x[n,i] + b[h,o]
            # col-tiled: 4 heads stacked on output partitions.
            for nq in range(N // 512):
                pt = ps_s.tile([P, 1024], F32, tag="s")
                # head-outer, chunk-inner: each head's accumulation chain
                # completes before the next head's start=True (safe even if
                # the HW first-matmul clear is bank-wide across partitions).
                for h in range(NH):
                    for c in range(IC):
                        nc.tensor.matmul(
                            pt[bass.ts(h, ATN), :512],
                            W_sb[:, c, h, :],
                            xT_sb[:, c, bass.ts(nq, 512)],
                            start=(c == 0),
                            stop=(c == IC - 1),
                            tile_position=(0, h * ATN),
                            skip_group_check=True,
                        )
                nc.scalar.activation(
                    xtT[:, bass.ts(nq, 512)], pt[:, :512], AF.Identity, bias=b_sb[:]
                )

            # xCT[32h+p, n] = sum_o C[h,p,o] xt[n,o]; diagonal 32x32 tiles.
            for nq in range(N // 512):
                pt = ps_s.tile([P, 1024], F32, tag="s")
                for h in range(NH):
                    nc.tensor.matmul(
                        pt[bass.ts(h, ATN), :512],
                        CT_sb[bass.ts(h, ATN), :],
                        xtT[bass.ts(h, ATN), bass.ts(nq, 512)],
                        start=True,
                        stop=True,
                        tile_position=(h * ATN, h * ATN),
                        skip_group_check=True,
                    )
                nc.scalar.copy(xCT[:, bass.ts(nq, 512)], pt[:, :512])

            # xt4[m_local, mc, f] = xt[mc*128+m_local, f]  (PE transpose)
            for mc in range(MC):
                pt = ps_s.tile([P, 1024], BF16, tag="s")
                nc.tensor.transpose(pt[:, :P], xtT[:, bass.ts(mc, P)], ident[:])
                nc.scalar.copy(xt4[:, mc, :], pt[:, :P])

            # --- main loop: n-half outer, m-chunks inner ---
            # Per (nh, mc): scores come out of PE in head-PAIR psum tiles
            # [128, 2, 512] so the two heads' K=32 matmuls run concurrently
            # in different PE row-groups while DVE still gets an FD=1024
            # multiply per instruction (adj broadcast over the pair dim).
            NHALF = N // 1024
            for nh in range(NHALF):
                po = ps_o.tile([P, 1024], F32, tag="po")
                # Seed the two accumulator banks with an explicit zeroing
                # matmul (K=1, zero weights) so every real outT matmul can
                # use start=False: correct regardless of whether the HW
                # first-matmul has_written clear is per-partition-slice or
                # bank-wide.
                for q in range(2):
                    nc.tensor.matmul(
                        po[:, bass.ts(q, 512)],
                        zrow[:, :P],
                        zrow[:, :512],
                        start=True,
                        stop=False,
                        skip_group_check=True,
                    )
                for mc in range(MC):
                    adjt = adjp.tile([P, 1024], F32, tag="adj")
                    nc.sync.dma_start(
                        adjt[:], adjT_d[bass.ts(mc, P), bass.ds(nh * 1024, 1024)]
                    )
                    zb = zp.tile([P, NH, 1024], F32, tag="z")
                    for hp in range(NH // 2):
                        for q in range(2):
                            s2 = ps_s.tile([P, 2, 512], F32, tag="s")
                            for j in range(2):
                                h = 2 * hp + j
                                nc.tensor.matmul(
                                    s2[:, j, :],
                                    xtT[bass.ts(h, ATN), bass.ts(mc, P)],
                                    xCT[
                                        bass.ts(h, ATN),
                                        bass.ds(nh * 1024 + q * 512, 512),
                                    ],
                                    start=True,
                                    stop=True,
                                    tile_position=(h * ATN, 0),
                                    skip_group_check=True,
                                )
                            nc.vector.tensor_tensor(
                                zb[:, bass.ds(2 * hp, 2), bass.ts(q, 512)],
                                s2[:],
                                adjt[:, None, bass.ts(q, 512)].to_broadcast(
                                    (P, 2, 512)
                                ),
                                ALU.mult,
                            )
                    alpha = alp.tile([P, NH, 1024], BF16, tag="alpha")
                    nc.scalar.activation(alpha[:], zb[:], AF.Tanh)
                    for q in range(2):
                        for h in range(NH):
                            nc.tensor.matmul(
                                po[bass.ts(h, ATN), bass.ts(q, 512)],
                                xt4[:, mc, bass.ts(h, ATN)],
                                alpha[:, h, bass.ts(q, 512)],
                                start=False,
                                stop=(mc == MC - 1 and h == NH - 1),
                                tile_position=(0, h * ATN),
                                skip_group_check=True,
                            )
                nc.scalar.copy(out_sb[:, bass.ds(nh * 1024, 1024)], po[:])

            nc.sync.dma_start(out_d[:], out_sb[:])

    nc.compile()
    return nc


def _get_graph():
    if "nc" not in _CACHE:
        _CACHE["nc"] = build_graph()
    return _CACHE["nc"]


def make_in_maps(x, adj, W, b, C):
    in_maps = []
    for core in range(NCORES):
        bb = core // 2
        hg = core % 2
        hs = slice(hg * NH, (hg + 1) * NH)
        in_maps.append(
            {
                "xT": np.ascontiguousarray(x[bb].T).astype(BF16_NP),
                "adjT": np.ascontiguousarray(adj[bb].T),
                "Wt": np.ascontiguousarray(
                    W[hs].reshape(NH, IC, P, ATN).transpose(2, 1, 0, 3)
                ).astype(BF16_NP),
                "CTt": np.ascontiguousarray(
                    C[hs].transpose(0, 2, 1).reshape(NH * ATN, ATN)
                ).astype(BF16_NP),
                "bias": np.ascontiguousarray(b[hs].reshape(P, 1)),
            }
        )
    return in_maps


LAST_RESULT = None


def kernel(x, adj, W, b, C):
    global LAST_RESULT
    x = np.asarray(x, dtype=np.float32)
    adj = np.asarray(adj, dtype=np.float32)
    W = np.asarray(W, dtype=np.float32)
    b = np.asarray(b, dtype=np.float32)
    C = np.asarray(C, dtype=np.float32)

    nc = _get_graph()
    in_maps = make_in_maps(x, adj, W, b, C)
    res = run_bass_kernel_spmd(nc, in_maps, core_ids=list(range(NCORES)))
    LAST_RESULT = res

    out = np.empty((B, N, H * ATN), dtype=np.float32)
    for core in range(NCORES):
        bb = core // 2
        hg = core % 2
        out[bb, :, hg * P : (hg + 1) * P] = res.results[core]["out"].T
    return out


# revision 12
# speedup vs baseline: 2.7667x; 1.0160x over previous
"""Trainium2 Bass kernel for per-head bilinear graph attention.

Reference computation (B=4, N=2048, IN=256, H=8, ATN=32):
    xt     = einsum('bni,hio->bhno', x, W) + b          # [B,H,N,32]
    xC     = einsum('bhno,hpo->bhnp', xt, C)            # [B,H,N,32]
    scores = einsum('bhnp,bhmp->bhnm', xC, xt)          # [B,H,N,N]
    alpha  = tanh(scores * adj[:,None])                 # [B,H,N,N]
    heads  = einsum('bhnm,bhmo->bhno', alpha, xt)       # [B,H,N,32]
    out    = concat heads on feature dim                # [B,N,256]

Sharding: 8 cores = 4 batches x 2 head-groups (4 heads each). Fully
data-parallel, no collectives. Each core computes out[b, :, hg*128:(hg+1)*128]
transposed ([128, 2048]); the host transposes back and concatenates.

Device-side layout is fully transposed ("T" = [feature/m, n]):
    xtT  [128(4h x 32o), 2048n]   stacked per-head xt^T (bias included)
    xCT  [128(4h x 32p), 2048n]   stacked per-head xC^T
    sT   [128m, n]     = scores[n, m]   (psum, per m-chunk per head)
    z    = sT * adjT   (adjT host-pretransposed so it is [m, n])
    alphaT = tanh(z)
    outT [128(4h x 32o), 2048n] accumulated in psum over 16 m-chunks

Engine budget per core (predicted): DVE mult ~153us (bottleneck), ACT
tanh ~125us, PE ~45us, DMA ~31us.
"""

import sys
import types

import numpy as np
import ml_dtypes

BF16_NP = ml_dtypes.bfloat16


def _ensure_axon_ntff_hook():
    """Provide antenv.axon_hooks if the image lacks it, so
    run_bass_kernel_spmd(trace=True) can capture NTFF profiles instead of
    crashing on the import. No-op when the real module exists."""
    try:
        import antenv.axon_hooks  # noqa: F401

        return
    except ImportError:
        pass
    mod = types.ModuleType("antenv.axon_hooks")
    _state = {"hook": None}
    mod.set_axon_ntff_profile_hook = lambda h: _state.__setitem__("hook", h)
    mod.get_axon_ntff_profile_hook = lambda: _state["hook"]
    sys.modules["antenv.axon_hooks"] = mod
    try:
        import antenv

        antenv.axon_hooks = mod
    except ImportError:
        pass
    try:
        from trn_agent_boot.trn_boot import _ntff_profile_via_ctypes

        mod.set_axon_ntff_profile_hook(
            _ntff_profile_via_ctypes("/opt/axon/libaxon_pjrt.so")
        )
    except Exception:
        pass


_ensure_axon_ntff_hook()

from concourse import bacc, mybir, tile
import concourse.bass as bass
from concourse.bass_utils import run_bass_kernel_spmd
from concourse.masks import make_identity

F32 = mybir.dt.float32
BF16 = mybir.dt.bfloat16
AF = mybir.ActivationFunctionType
ALU = mybir.AluOpType

P = 128
B, N, IN_DIM, H, ATN = 4, 2048, 256, 8, 32
NH = 4                # heads per core
NCORES = 8
MC = N // P           # 16 m-chunks
IC = IN_DIM // P      # 2 contraction chunks for the input projection

_CACHE = {}


def build_graph():
    nc = bacc.Bacc("TRN2", target_bir_lowering=False, debug=False)

    xT_d = nc.dram_tensor("xT", [IN_DIM, N], BF16, kind="ExternalInput")
    adjT_d = nc.dram_tensor("adjT", [N, N], F32, kind="ExternalInput")
    W_d = nc.dram_tensor("Wt", [P, IC, NH, ATN], BF16, kind="ExternalInput")
    CT_d = nc.dram_tensor("CTt", [P, ATN], BF16, kind="ExternalInput")
    b_d = nc.dram_tensor("bias", [P, 1], F32, kind="ExternalInput")
    out_d = nc.dram_tensor("out", [P, N], F32, kind="ExternalOutput")

    with tile.TileContext(nc) as tc:
        with (
            tc.tile_pool(name="const", bufs=1) as cp,
            tc.tile_pool(name="adj", bufs=4) as adjp,
            tc.tile_pool(name="z", bufs=3) as zp,
            tc.tile_pool(name="alpha", bufs=2) as alp,
            tc.tile_pool(name="ps_o", bufs=1, space="PSUM") as ps_o,
            tc.tile_pool(name="ps_s", bufs=3, space="PSUM") as ps_s,
        ):
            ident = cp.tile([P, P], BF16)
            make_identity(nc, ident[:])

            xT_sb = cp.tile([P, IC, N], BF16)
            nc.sync.dma_start(xT_sb[:], xT_d[:].rearrange("(c p) n -> p c n", p=P))
            W_sb = cp.tile([P, IC, NH, ATN], BF16)
            nc.sync.dma_start(W_sb[:], W_d[:])
            CT_sb = cp.tile([P, ATN], BF16)
            nc.sync.dma_start(CT_sb[:], CT_d[:])
            b_sb = cp.tile([P, 1], F32)
            nc.sync.dma_start(b_sb[:], b_d[:])

            xtT = cp.tile([P, N], BF16)
            xCT = cp.tile([P, N], BF16)
            xt4 = cp.tile([P, MC, P], BF16)
            out_sb = cp.tile([P, N], F32)
            zrow = cp.tile([1, 512], BF16)
            nc.vector.memset(zrow[:], 0.0)

            # --- prologue (reuses the ps_s slots) ---
            # xtT[32h+o, n] = sum_i W[h,i,o] x[n,i] + b[h,o]
            # col-tiled: 4 heads stacked on output partitions.
            for nq in range(N // 512):
                pt = ps_s.tile([P, 1024], F32, tag="s")
                # head-outer, chunk-inner: each head's accumulation chain
                # completes before the next head's start=True (safe even if
                # the HW first-matmul clear is bank-wide across partitions).
                for h in range(NH):
                    for c in range(IC):
                        nc.tensor.matmul(
                            pt[bass.ts(h, ATN), :512],
                            W_sb[:, c, h, :],
                            xT_sb[:, c, bass.ts(nq, 512)],
                            start=(c == 0),
                            stop=(c == IC - 1),
                            tile_position=(0, h * ATN),
                            skip_group_check=True,
                        )
                nc.scalar.activation(
                    xtT[:, bass.ts(nq, 512)], pt[:, :512], AF.Identity, bias=b_sb[:]
                )

            # xCT[32h+p, n] = sum_o C[h,p,o] xt[n,o]; diagonal 32x32 tiles.
            for nq in range(N // 512):
                pt = ps_s.tile([P, 1024], F32, tag="s")
                for h in range(NH):
                    nc.tensor.matmul(
                        pt[bass.ts(h, ATN), :512],
                        CT_sb[bass.ts(h, ATN), :],
                        xtT[bass.ts(h, ATN), bass.ts(nq, 512)],
                        start=True,
                        stop=True,
                        tile_position=(h * ATN, h * ATN),
                        skip_group_check=True,
                    )
                nc.scalar.copy(xCT[:, bass.ts(nq, 512)], pt[:, :512])

            # --- main loop: n-half outer, m-chunks inner ---
            # Per (nh, mc): scores come out of PE in head-PAIR psum tiles
            # [128, 2, 512] so the two heads' K=32 matmuls run concurrently
            # in different PE row-groups while DVE still gets an FD=1024
            # multiply per instruction (adj broadcast over the pair dim).
            NHALF = N // 1024
            for nh in range(NHALF):
                po = ps_o.tile([P, 1024], F32, tag="po")
                # Seed the two accumulator banks with an explicit zeroing
                # matmul (K=1, zero weights) so every real outT matmul can
                # use start=False: correct regardless of whether the HW
                # first-matmul has_written clear is per-partition-slice or
                # bank-wide.
                for q in range(2):
                    nc.tensor.matmul(
                        po[:, bass.ts(q, 512)],
                        zrow[:, :P],
                        zrow[:, :512],
                        start=True,
                        stop=False,
                        skip_group_check=True,
                    )
                for mc in range(MC):
                    if nh == 0 and mc % 4 == 0:
                        # xt4[m_local, mc, f] = xt[mc*128+m_local, f]: transpose
                        # 4 m-chunks of xtT through one psum tile (PE), copied
                        # out on ACT (which has slack mid-loop).
                        g = mc // 4
                        pt = ps_s.tile([P, 4, P], BF16, tag="s")
                        for k in range(4):
                            nc.tensor.transpose(
                                pt[:, k, :], xtT[:, bass.ts(4 * g + k, P)], ident[:]
                            )
                        nc.scalar.copy(xt4[:, bass.ds(4 * g, 4), :], pt[:])
                    adjt = adjp.tile([P, 1024], F32, tag="adj")
                    nc.sync.dma_start(
                        adjt[:], adjT_d[bass.ts(mc, P), bass.ds(nh * 1024, 1024)]
                    )
                    zb = zp.tile([P, NH, 1024], F32, tag="z")
                    for hp in range(NH // 2):
                        for q in range(2):
                            s2 = ps_s.tile([P, 2, 512], F32, tag="s")
                            for j in range(2):
                                h = 2 * hp + j
                                nc.tensor.matmul(
                                    s2[:, j, :],
                                    xtT[bass.ts(h, ATN), bass.ts(mc, P)],
                                    xCT[
                                        bass.ts(h, ATN),
                                        bass.ds(nh * 1024 + q * 512, 512),
                                    ],
                                    start=True,
                                    stop=True,
                                    tile_position=(h * ATN, 0),
                                    skip_group_check=True,
                                )
                            nc.vector.tensor_tensor(
                                zb[:, bass.ds(2 * hp, 2), bass.ts(q, 512)],
                                s2[:],
                                adjt[:, None, bass.ts(q, 512)].to_broadcast(
                                    (P, 2, 512)
                                ),
                                ALU.mult,
                            )
                    alpha = alp.tile([P, NH, 1024], BF16, tag="alpha")
                    nc.scalar.activation(alpha[:], zb[:], AF.Tanh)
                    for q in range(2):
                        for h in range(NH):
                            nc.tensor.matmul(
                                po[bass.ts(h, ATN), bass.ts(q, 512)],
                                xt4[:, mc, bass.ts(h, ATN)],
                                alpha[:, h, bass.ts(q, 512)],
                                start=False,
                                stop=(mc == MC - 1 and h == NH - 1),
                                tile_position=(0, h * ATN),
                                skip_group_check=True,
                            )
                nc.scalar.copy(out_sb[:, bass.ds(nh * 1024, 1024)], po[:])
                nc.sync.dma_start(
                    out_d[:, bass.ds(nh * 1024, 1024)],
                    out_sb[:, bass.ds(nh * 1024, 1024)],
                )

    nc.compile()
    return nc


def _get_graph():
    if "nc" not in _CACHE:
        _CACHE["nc"] = build_graph()
    return _CACHE["nc"]


def make_in_maps(x, adj, W, b, C):
    in_maps = []
    for core in range(NCORES):
        bb = core // 2
        hg = core % 2
        hs = slice(hg * NH, (hg + 1) * NH)
        in_maps.append(
            {
                "xT": np.ascontiguousarray(x[bb].T).astype(BF16_NP),
                "adjT": np.ascontiguousarray(adj[bb].T),
                "Wt": np.ascontiguousarray(
                    W[hs].reshape(NH, IC, P, ATN).transpose(2, 1, 0, 3)
                ).astype(BF16_NP),
                "CTt": np.ascontiguousarray(
                    C[hs].transpose(0, 2, 1).reshape(NH * ATN, ATN)
                ).astype(BF16_NP),
                "bias": np.ascontiguousarray(b[hs].reshape(P, 1)),
            }
        )
    return in_maps


LAST_RESULT = None


def kernel(x, adj, W, b, C):
    global LAST_RESULT
    x = np.asarray(x, dtype=np.float32)
    adj = np.asarray(adj, dtype=np.float32)
    W = np.asarray(W, dtype=np.float32)
    b = np.asarray(b, dtype=np.float32)
    C = np.asarray(C, dtype=np.float32)

    nc = _get_graph()
    in_maps = make_in_maps(x, adj, W, b, C)
    res = run_bass_kernel_spmd(nc, in_maps, core_ids=list(range(NCORES)))
    LAST_RESULT = res

    out = np.empty((B, N, H * ATN), dtype=np.float32)
    for core in range(NCORES):
        bb = core // 2
        hg = core % 2
        out[bb, :, hg * P : (hg + 1) * P] = res.results[core]["out"].T
    return out
